# revision 7
# baseline (speedup 1.0000x reference)
# Trainium2 Bass kernel for CollaborativeRNNModel (GRU recurrence + big logits matmul).
#
# Sharding: the 256-step GRU recurrence is sequential and per-step cross-core
# collectives are far too slow (>=5us floor per call), so every core runs the
# full-batch (B=32) recurrence redundantly; the 10001-wide logits matmul is
# sharded across the 8 cores by vocab columns (1251 each, zero-padded).
#
# Recurrence layout (per step):
#   h "packed"  [128, 256] f32 : partition 32j+u <-> h[u, 256j+c]  (j=0..3)
#   h^T tiles   [128, 256] bf16: cols 32k+u      <-> h[u, 128k+p]  (k=0..7)
# mm1/mm2 use h^T tiles as the PE stationary operand with col-tiling
# (tile_position=(0,32j)) so the 4 groups' outputs land at partitions 32j+u,
# matching the packed layout, and all 128 PE columns stay busy.
import numpy as np
import ml_dtypes
from contextlib import ExitStack

import concourse.bass as bass
import concourse.tile as tile
import concourse.mybir as mybir
from concourse import bacc
from concourse.bass_utils import run_bass_kernel_spmd
from concourse.masks import make_identity

F32 = mybir.dt.float32
BF16 = mybir.dt.bfloat16
I32 = mybir.dt.int32

B, H, V = 32, 1024, 10001
NCORES = 8
KC = H // 128          # 8 k-chunks of the hidden dim
G = 4                  # col-tile groups
GW = H // G            # 256: per-group width of H-wide quantities
VSH = 1256             # vocab cols per core (8*1256 = 10048 >= 10001, mult of 8)
SB = 16                # steps staged in SBUF before statesT flush to DRAM


def build_program(T):
    PAIRS = B * T
    nc = bacc.Bacc("TRN2", target_bir_lowering=False, debug=False,
                   enable_asserts=False, num_devices=NCORES)

    idx_d = nc.dram_tensor("idx", [B, T], I32, kind="ExternalInput").ap()
    h0_d = nc.dram_tensor("h0", [B, H], F32, kind="ExternalInput").ap()
    erub_d = nc.dram_tensor("erub", [V, 2 * H], BF16, kind="ExternalInput").ap()
    ecb_d = nc.dram_tensor("ecb", [V, H], BF16, kind="ExternalInput").ap()
    wru_d = nc.dram_tensor("wru", [H, 2 * H], BF16, kind="ExternalInput").ap()
    wc_d = nc.dram_tensor("wc", [H, H], BF16, kind="ExternalInput").ap()
    wout_d = nc.dram_tensor("wout", [H, VSH], BF16, kind="ExternalInput").ap()

    hfin_d = nc.dram_tensor("hfin", [B, H], F32, kind="ExternalOutput").ap()
    logits_d = nc.dram_tensor("logits", [PAIRS, VSH], F32, kind="ExternalOutput").ap()

    with tile.TileContext(nc) as tc, ExitStack() as ctx:
        const = ctx.enter_context(tc.tile_pool(name="const", bufs=1))
        dram = ctx.enter_context(tc.tile_pool(name="dram", bufs=1, space="DRAM"))

        statesT = dram.tile([KC * 128, PAIRS], BF16)

        # resident weights (moving operands), k-chunk major in the free dim
        wru_sb = const.tile([128, KC * 2 * H], BF16, tag="wru")
        wc_sb = const.tile([128, KC * H], BF16, tag="wc")
        wout_sb = const.tile([128, KC * VSH], BF16, tag="wout")
        for k in range(KC):
            nc.sync.dma_start(wru_sb[:, k * 2 * H:(k + 1) * 2 * H],
                              wru_d[k * 128:(k + 1) * 128, :])
            nc.sync.dma_start(wc_sb[:, k * H:(k + 1) * H],
                              wc_d[k * 128:(k + 1) * 128, :])
            nc.sync.dma_start(wout_sb[:, k * VSH:(k + 1) * VSH],
                              wout_d[k * 128:(k + 1) * 128, :])

        idx_sb = const.tile([B, T], I32, tag="idx")
        nc.sync.dma_start(idx_sb[:], idx_d[:])

        ident = const.tile([128, 128], F32, tag="ident")
        make_identity(nc, ident[:])

        def idn(j):
            return ident[32 * j:32 * (j + 1), 32 * j:32 * (j + 1)]

        # transposed-state tiles live interleaved: tile k at cols cb(k)..cb(k)+32
        def cb(k):
            return (k % 2) * 128 + 32 * (k // 2)

        # initial state, both layouts
        h0_sb = const.tile([B, H], F32, tag="h0sb")
        nc.sync.dma_start(h0_sb[:], h0_d[:])
        hT0 = const.tile([128, KC * B], BF16, tag="hT0")

        with tc.tile_pool(name="psI", bufs=1, space="PSUM") as psI:
            psT0 = psI.tile([128, KC * B], F32)
            for k in range(KC):
                nc.tensor.transpose(psT0[:, cb(k):cb(k) + B],
                                    h0_sb[:, k * 128:(k + 1) * 128], idn(0))
            nc.vector.tensor_copy(hT0[:], psT0[:])

        phase_a = ExitStack()
        hpk = phase_a.enter_context(tc.tile_pool(name="hpk", bufs=2))
        xrt = phase_a.enter_context(tc.tile_pool(name="xrt", bufs=4))
        work = phase_a.enter_context(tc.tile_pool(name="work", bufs=2))
        stp = phase_a.enter_context(tc.tile_pool(name="stp", bufs=2))
        psum = phase_a.enter_context(tc.tile_pool(name="psum", bufs=2, space="PSUM"))

        h_pk = hpk.tile([128, GW], F32, tag="h")
        for j in range(G):
            nc.sync.dma_start(h_pk[32 * j:32 * (j + 1), :],
                              h0_d[:, GW * j:GW * (j + 1)])

        st_cur = stp.tile([128, SB * KC * B], BF16, tag="st")
        st_prev = None

        for t in range(T):
            s = t % SB
            # h^T tiles for this step's stationary operand
            if t == 0:
                hT = hT0[:]
            else:
                sp = (t - 1) % SB
                src = st_cur if sp != SB - 1 else st_prev
                hT = src[:, sp * KC * B:(sp + 1) * KC * B]

            # --- gather x for this step (prefetched via pool bufs) ---
            xru_t = xrt.tile([128, 2 * GW], BF16, tag="xru")
            xc_t = xrt.tile([128, GW], BF16, tag="xc")
            for j in range(G):
                nc.gpsimd.indirect_dma_start(
                    out=xru_t[32 * j:32 * (j + 1), 0:GW], out_offset=None,
                    in_=erub_d[:],
                    in_offset=bass.IndirectOffsetOnAxis(ap=idx_sb[:, t:t + 1], axis=0),
                    element_offset=GW * j)
                nc.gpsimd.indirect_dma_start(
                    out=xru_t[32 * j:32 * (j + 1), GW:2 * GW], out_offset=None,
                    in_=erub_d[:],
                    in_offset=bass.IndirectOffsetOnAxis(ap=idx_sb[:, t:t + 1], axis=0),
                    element_offset=H + GW * j)
                nc.gpsimd.indirect_dma_start(
                    out=xc_t[32 * j:32 * (j + 1), :], out_offset=None,
                    in_=ecb_d[:],
                    in_offset=bass.IndirectOffsetOnAxis(ap=idx_sb[:, t:t + 1], axis=0),
                    element_offset=GW * j)

            # --- mm1: ru_pre = h @ W_ru; W_ru cols pre-interleaved so group j's
            # r and u halves are one contiguous N=512 chunk ---
            ps_ru = psum.tile([128, 2 * GW], F32, tag="ps_ru")
            for k in range(KC):
                lhsT = hT[:, cb(k):cb(k) + B]
                for j in range(G):
                    nc.tensor.matmul(
                        ps_ru[32 * j:32 * (j + 1), :], lhsT,
                        wru_sb[:, k * 2 * H + 512 * j: k * 2 * H + 512 * (j + 1)],
                        start=(k == 0), stop=(k == KC - 1),
                        skip_group_check=True, tile_position=(0, 32 * j))

            pre_ru = work.tile([128, 2 * GW], F32, tag="pre_ru")
            nc.vector.tensor_add(pre_ru[:], ps_ru[:], xru_t[:])
            ru = work.tile([128, 2 * GW], F32, tag="ru")
            nc.scalar.activation(ru[:], pre_ru[:],
                                 mybir.ActivationFunctionType.Sigmoid)

            # --- rh = r * h, then transpose to rh^T tiles ---
            rh = work.tile([128, GW], F32, tag="rh")
            nc.vector.tensor_mul(rh[:], ru[:, 0:GW], h_pk[:])

            ps_t1 = psum.tile([128, KC * B], F32, tag="ps_t1")
            nc.tensor.transpose(ps_t1[:, 0:128], rh[:, 0:128], ident[:])
            nc.tensor.transpose(ps_t1[:, 128:256], rh[:, 128:256], ident[:])
            rhT = work.tile([128, KC * B], BF16, tag="rhT")
            nc.vector.tensor_copy(rhT[:], ps_t1[:])

            # --- mm2: c_pre = (r*h) @ W_c (col-tiled) ---
            ps_c = psum.tile([128, GW], F32, tag="ps_c")
            for k in range(KC):
                lhsT = rhT[:, cb(k):cb(k) + B]
                for j in range(G):
                    nc.tensor.matmul(
                        ps_c[32 * j:32 * (j + 1), :], lhsT,
                        wc_sb[:, k * H + GW * j: k * H + GW * (j + 1)],
                        start=(k == 0), stop=(k == KC - 1),
                        skip_group_check=True, tile_position=(0, 32 * j))

            pre_c = work.tile([128, GW], F32, tag="pre_c")
            nc.vector.tensor_add(pre_c[:], ps_c[:], xc_t[:])
            c_sb = work.tile([128, GW], F32, tag="c")
            nc.scalar.activation(c_sb[:], pre_c[:],
                                 mybir.ActivationFunctionType.Tanh)

            # --- h' = u*h + (1-u)*c = (h-c)*u + c ---
            d_sb = work.tile([128, GW], F32, tag="d")
            nc.vector.tensor_sub(d_sb[:], h_pk[:], c_sb[:])
            e_sb = work.tile([128, GW], F32, tag="e")
            nc.vector.tensor_mul(e_sb[:], d_sb[:], ru[:, GW:2 * GW])
            h_new = hpk.tile([128, GW], F32, tag="h")
            nc.vector.tensor_add(h_new[:], e_sb[:], c_sb[:])

            # --- transpose h' into the statesT staging buffer (bf16) ---
            ps_t2 = psum.tile([128, KC * B], F32, tag="ps_t2")
            nc.tensor.transpose(ps_t2[:, 0:128], h_new[:, 0:128], ident[:])
            nc.tensor.transpose(ps_t2[:, 128:256], h_new[:, 128:256], ident[:])
            nc.vector.tensor_copy(st_cur[:, s * KC * B:(s + 1) * KC * B], ps_t2[:])

            h_pk = h_new

            if s == SB - 1 or t == T - 1:
                blk = t // SB
                nsteps = s + 1
                st3 = st_cur[:].rearrange("p (s two jj u) -> p s two jj u",
                                          s=SB, two=2, jj=G)
                for k in range(KC):
                    nc.sync.dma_start(
                        statesT[k * 128:(k + 1) * 128,
                                blk * SB * B: blk * SB * B + nsteps * B],
                        st3[:, 0:nsteps, k % 2, k // 2, :])
                st_prev = st_cur
                st_cur = stp.tile([128, SB * KC * B], BF16, tag="st")

        for j in range(G):
            nc.sync.dma_start(hfin_d[:, GW * j:GW * (j + 1)],
                              h_pk[32 * j:32 * (j + 1), :])
        phase_a.close()

        # ---------------- phase B: logits = states @ W_out (vocab shard) -------
        NCH = [(n, min(512, VSH - n)) for n in range(0, VSH, 512)]
        with tc.tile_pool(name="lst", bufs=3) as lst, \
             tc.tile_pool(name="lout", bufs=2) as lout, \
             tc.tile_pool(name="lps", bufs=2, space="PSUM") as lps:
            for m in range(PAIRS // 128):
                stT = lst.tile([128, KC * 128], BF16, tag="stT")
                for k in range(KC):
                    nc.sync.dma_start(stT[:, k * 128:(k + 1) * 128],
                                      statesT[k * 128:(k + 1) * 128,
                                              m * 128:(m + 1) * 128])
                ps_l = lps.tile([128, VSH], F32, tag="ps_l")
                for (n0, nsz) in NCH:
                    for k in range(KC):
                        nc.tensor.matmul(
                            ps_l[:, n0:n0 + nsz], stT[:, k * 128:(k + 1) * 128],
                            wout_sb[:, k * VSH + n0: k * VSH + n0 + nsz],
                            start=(k == 0), stop=(k == KC - 1))
                out_sb = lout.tile([128, VSH], F32, tag="out_sb")
                nc.vector.tensor_copy(out_sb[:], ps_l[:])
                # pair p = 32*s + u in this block maps to logits row u*T + (4m+s)
                dst = logits_d.rearrange("(u t) v -> t u v", u=B)[4 * m:4 * (m + 1)]
                nc.sync.dma_start(dst, out_sb[:])

    nc.compile()
    return nc


_PROG_CACHE = {}


def _get_program(T):
    if T not in _PROG_CACHE:
        _PROG_CACHE[T] = build_program(T)
    return _PROG_CACHE[T]


def interleave_wru(W_ru):
    # col order: for j in 0..3: [r cols 256j:256(j+1) | u cols H+256j:H+256(j+1)]
    blocks = []
    for j in range(G):
        blocks.append(W_ru[:, GW * j:GW * (j + 1)])
        blocks.append(W_ru[:, H + GW * j:H + GW * (j + 1)])
    return np.ascontiguousarray(np.concatenate(blocks, axis=1))


def kernel(items, h0, E_ru, W_ru, b_ru, E_c, W_c, b_c, W_out):
    T = int(np.asarray(items).shape[1])
    items = np.asarray(items).astype(np.int32)
    h0 = np.asarray(h0, dtype=np.float32)
    bf = ml_dtypes.bfloat16
    erub = (np.asarray(E_ru, np.float32) + np.asarray(b_ru, np.float32)).astype(bf)
    ecb = (np.asarray(E_c, np.float32) + np.asarray(b_c, np.float32)).astype(bf)
    wru = interleave_wru(np.asarray(W_ru, np.float32)).astype(bf)
    wc = np.asarray(W_c, np.float32).astype(bf)
    wout_full = np.asarray(W_out, np.float32)
    wout_pad = np.zeros((H, NCORES * VSH), np.float32)
    wout_pad[:, :V] = wout_full
    wout_pad = wout_pad.astype(bf)

    nc = _get_program(T)
    in_maps = []
    for c in range(NCORES):
        in_maps.append({
            "idx": items, "h0": h0, "erub": erub, "ecb": ecb,
            "wru": wru, "wc": wc,
            "wout": np.ascontiguousarray(wout_pad[:, c * VSH:(c + 1) * VSH]),
        })
    res = run_bass_kernel_spmd(nc, in_maps, core_ids=list(range(NCORES)))
    h_final = res.results[0]["hfin"]
    logits = np.concatenate([res.results[c]["logits"] for c in range(NCORES)],
                            axis=1)[:, :V]
    return h_final, logits


# revision 13
# speedup vs baseline: 5.5388x; 5.5388x over previous
# Trainium2 Bass kernel for CollaborativeRNNModel (GRU recurrence + big logits matmul).
#
# Sharding: the 256-step GRU recurrence is sequential and per-step cross-core
# collectives are far too slow (>=5us floor per call), so every core runs the
# full-batch (B=32) recurrence redundantly; the 10001-wide logits matmul is
# sharded across the 8 cores by vocab columns (1256 each, zero-padded).
#
# Per-step layouts:
#   h "packed"  [128, 256] f32 : partition 32j+u <-> h[u, 256j+c]  (j=0..3)
#   h^T tiles   [128, 32]  bf16: staged per 4-step block at cols k*128+sl*32+u
# mm1/mm2 use h^T tiles as the PE stationary operand with col-tiling
# (tile_position=(0,32j)) so group j's output lands at partitions 32j+u,
# matching the packed layout, and all 128 PE columns stay busy.
#
# x-gather: one indirect DMA per 4 steps pulls 128 full rows of the
# host-concatenated [V, 3H] table [E_ru+b_ru | E_c+b_c] (row -> partition
# 32*(t%4)+u), then 3 small SBUF->SBUF HWDGE DMAs per step repack into the
# packed layout. Logits matmuls are emitted between recurrence matmuls to
# fill PE wait-gaps; their PSUM->SBUF copies run on the mostly-idle ScalarE.
import numpy as np
import ml_dtypes
from contextlib import ExitStack

import concourse.bass as bass
import concourse.tile as tile
import concourse.mybir as mybir
from concourse import bacc
from concourse.bass_utils import run_bass_kernel_spmd
from concourse.masks import make_identity

F32 = mybir.dt.float32
BF16 = mybir.dt.bfloat16
I32 = mybir.dt.int32

B, H, V = 32, 1024, 10001
NCORES = 8
KC = H // 128          # 8 k-chunks of the hidden dim
G = 4                  # col-tile groups
GW = H // G            # 256: per-group width of H-wide quantities
VSH = 1256             # vocab cols per core (8*1256 = 10048 >= 10001)
LCH = [(0, 512), (512, 512), (1024, VSH - 1024)]  # logits n-chunks


def build_program(T, reps=1):
    assert T % 4 == 0
    PAIRS = B * T
    TB = T // 4            # 4-step blocks
    nc = bacc.Bacc("TRN2", target_bir_lowering=False, debug=False,
                   enable_asserts=False, num_devices=NCORES)

    idx_d = nc.dram_tensor("idx", [B, T], I32, kind="ExternalInput").ap()
    h0_d = nc.dram_tensor("h0", [B, H], F32, kind="ExternalInput").ap()
    # eall = [E_ru + b_ru | E_c + b_c] : [V, 3H]
    eall_d = nc.dram_tensor("eall", [V, 3 * H], BF16, kind="ExternalInput").ap()
    wru_d = nc.dram_tensor("wru", [H, 2 * H], BF16, kind="ExternalInput").ap()
    wc_d = nc.dram_tensor("wc", [H, H], BF16, kind="ExternalInput").ap()
    wout_d = nc.dram_tensor("wout", [H, VSH], BF16, kind="ExternalInput").ap()

    hfin_d = nc.dram_tensor("hfin", [B, H], F32, kind="ExternalOutput").ap()
    logits_d = nc.dram_tensor("logits", [PAIRS, VSH], F32, kind="ExternalOutput").ap()

    with tile.TileContext(nc) as tc, ExitStack() as ctx:
        const = ctx.enter_context(tc.tile_pool(name="const", bufs=1))

        wru_sb = const.tile([128, KC * 2 * H], BF16, tag="wru")
        wc_sb = const.tile([128, KC * H], BF16, tag="wc")
        wout_sb = const.tile([128, KC * VSH], BF16, tag="wout")
        for k in range(KC):
            nc.sync.dma_start(wru_sb[:, k * 2 * H:(k + 1) * 2 * H],
                              wru_d[k * 128:(k + 1) * 128, :])
            nc.sync.dma_start(wc_sb[:, k * H:(k + 1) * H],
                              wc_d[k * 128:(k + 1) * 128, :])
            nc.sync.dma_start(wout_sb[:, k * VSH:(k + 1) * VSH],
                              wout_d[k * 128:(k + 1) * 128, :])

        # index tile for the batched gather: idx4[32*tl+u, m] = items[u, 4m+tl]
        idx4 = const.tile([128, TB], I32, tag="idx4")
        for tl in range(4):
            nc.sync.dma_start(
                idx4[32 * tl:32 * (tl + 1), :],
                idx_d[:].rearrange("u (m tl) -> u m tl", tl=4)[:, :, tl])

        ident = const.tile([128, 128], F32, tag="ident")
        make_identity(nc, ident[:])

        # initial state, both layouts
        h0_sb = const.tile([B, H], F32, tag="h0sb")
        nc.sync.dma_start(h0_sb[:], h0_d[:])
        hT0 = const.tile([128, KC * B], BF16, tag="hT0")
        with tc.tile_pool(name="psI", bufs=1, space="PSUM") as psI:
            psT0 = psI.tile([128, KC * B], F32)
            for k in range(KC):
                nc.tensor.transpose(psT0[:, k * B:(k + 1) * B],
                                    h0_sb[:, k * 128:(k + 1) * 128],
                                    ident[0:B, 0:B])
            nc.vector.tensor_copy(hT0[:], psT0[:])

        phase_a = ExitStack()
        hpk = phase_a.enter_context(tc.tile_pool(name="hpk", bufs=2))
        gat = phase_a.enter_context(tc.tile_pool(name="gat", bufs=3))
        xrt = phase_a.enter_context(tc.tile_pool(name="xrt", bufs=3))
        work = phase_a.enter_context(tc.tile_pool(name="work", bufs=2))
        stp = phase_a.enter_context(tc.tile_pool(name="stp", bufs=4))
        psum = phase_a.enter_context(tc.tile_pool(name="psum", bufs=1, space="PSUM"))
        pst = phase_a.enter_context(tc.tile_pool(name="pst", bufs=2, space="PSUM"))
        lps = phase_a.enter_context(tc.tile_pool(name="lps", bufs=2, space="PSUM"))
        lout = phase_a.enter_context(tc.tile_pool(name="lout", bufs=2))

        h_pk = hpk.tile([128, GW], F32, tag="h")
        for j in range(G):
            nc.sync.dma_start(h_pk[32 * j:32 * (j + 1), :],
                              h0_d[:, GW * j:GW * (j + 1)])

        st_blocks = {}   # block index -> tile [128, KC*128] bf16

        # ---- deferred logits work: thunks emitted into PE wait-gaps ----
        pending = []

        def queue_logits_block(m):
            stt = st_blocks[m]
            out_sb = lout.tile([128, VSH], F32, tag="lo")
            for (n0, nsz) in LCH:
                ps_l = lps.tile([128, 512], F32, tag="lps")
                for k in range(KC):
                    def mm(k=k, n0=n0, nsz=nsz, ps_l=ps_l, stt=stt):
                        nc.tensor.matmul(
                            ps_l[:, 0:nsz], stt[:, k * 128:(k + 1) * 128],
                            wout_sb[:, k * VSH + n0: k * VSH + n0 + nsz],
                            start=(k == 0), stop=(k == KC - 1))
                    pending.append(mm)

                def cp(n0=n0, nsz=nsz, ps_l=ps_l, out_sb=out_sb):
                    nc.scalar.copy(out_sb[:, n0:n0 + nsz], ps_l[:, 0:nsz])
                pending.append(cp)

            def dma(m=m, out_sb=out_sb):
                dst = logits_d.rearrange("(u t) v -> t u v", u=B)[4 * m:4 * (m + 1)]
                nc.sync.dma_start(dst, out_sb[:])
            pending.append(dma)

        def drain(n):
            for _ in range(min(n, len(pending))):
                pending.pop(0)()

        for rep_t in range(reps * T):
            t = rep_t % T
            m, sl = t // 4, t % 4

            if sl == 0:
                # gather 4 steps x 32 users of full [3H] table rows;
                # row (tl, u) -> partition 32*tl+u
                g4 = gat.tile([128, 3 * H], BF16, tag="g4")
                nc.gpsimd.indirect_dma_start(
                    out=g4[:], out_offset=None,
                    in_=eall_d[:],
                    in_offset=bass.IndirectOffsetOnAxis(ap=idx4[:, m:m + 1], axis=0),
                )
                st_blocks[m] = stp.tile([128, KC * 128], BF16, tag="st", name="stb")
                if m >= 3:
                    st_blocks.pop(m - 3, None)

            # repack this step's rows into packed layout (HWDGE, off-chain)
            xru_t = xrt.tile([128, 2 * GW], BF16, tag="xru")
            xc_t = xrt.tile([128, GW], BF16, tag="xc")
            gsrc = g4[32 * sl:32 * (sl + 1), :]
            # dst partition 32j+u: col c <- gsrc[u, 256j + c]         (r part)
            #                  col 256+c <- gsrc[u, 1024 + 256j + c]  (u part)
            #   xc:             col c <- gsrc[u, 2048 + 256j + c]     (c part)
            gh = gsrc.rearrange("u (half c) -> u half c", half=12)
            for j in range(G):
                eng = nc.scalar if j % 2 else nc.sync
                eng.dma_start(xru_t[32 * j:32 * (j + 1), :],
                              gh[:, j:j + 5:4, :])
                eng.dma_start(xc_t[32 * j:32 * (j + 1), :],
                              gh[:, 8 + j:9 + j, :])

            # h^T stationary tiles for this step
            if rep_t == 0:
                def hT(k):
                    return hT0[:, k * B:(k + 1) * B]
            else:
                pt = (rep_t - 1) % T
                pm, psl = pt // 4, pt % 4
                pblk = st_blocks[pm]

                def hT(k, pblk=pblk, psl=psl):
                    return pblk[:, k * 128 + psl * B: k * 128 + (psl + 1) * B]

            # --- mm1 r-phase: r_pre = h @ W_ru[:, :H] (col-tiled) ---
            ps_r = psum.tile([128, GW], F32, tag="ps_r")
            for k in range(KC):
                for j in range(G):
                    nc.tensor.matmul(
                        ps_r[32 * j:32 * (j + 1), :], hT(k),
                        wru_sb[:, k * 2 * H + GW * j: k * 2 * H + GW * (j + 1)],
                        start=(k == 0), stop=(k == KC - 1),
                        skip_group_check=True, tile_position=(0, 32 * j))
            # --- mm1 u-phase (PE busy while DVE/ACT chew on r) ---
            ps_u = psum.tile([128, GW], F32, tag="ps_u")
            for k in range(KC):
                for j in range(G):
                    nc.tensor.matmul(
                        ps_u[32 * j:32 * (j + 1), :], hT(k),
                        wru_sb[:, k * 2 * H + H + GW * j: k * 2 * H + H + GW * (j + 1)],
                        start=(k == 0), stop=(k == KC - 1),
                        skip_group_check=True, tile_position=(0, 32 * j))

            pre_r = work.tile([128, GW], F32, tag="pre_r")
            nc.vector.tensor_add(pre_r[:], ps_r[:], xru_t[:, 0:GW])
            r_sb = work.tile([128, GW], F32, tag="r")
            nc.scalar.activation(r_sb[:], pre_r[:],
                                 mybir.ActivationFunctionType.Sigmoid)
            rh = work.tile([128, GW], F32, tag="rh")
            nc.vector.tensor_mul(rh[:], r_sb[:], h_pk[:])

            # u-side, off the critical chain
            pre_u = work.tile([128, GW], F32, tag="pre_u")
            nc.vector.tensor_add(pre_u[:], ps_u[:], xru_t[:, GW:2 * GW])
            u_sb = work.tile([128, GW], F32, tag="u")
            nc.scalar.activation(u_sb[:], pre_u[:],
                                 mybir.ActivationFunctionType.Sigmoid)

            # --- transpose rh -> rh^T tiles ---
            ps_t1 = pst.tile([128, KC * B], F32, tag="ps_t")
            nc.tensor.transpose(ps_t1[:, 0:128], rh[:, 0:128], ident[:])
            nc.tensor.transpose(ps_t1[:, 128:256], rh[:, 128:256], ident[:])
            rhT = work.tile([128, KC * B], BF16, tag="rhT")
            nc.vector.tensor_copy(rhT[:], ps_t1[:])

            drain(2)

            # --- mm2: c_pre = (r*h) @ W_c (col-tiled) ---
            # rh^T tile k sits at cols (k%2)*128 + 32*(k//2) of rhT
            ps_c = psum.tile([128, GW], F32, tag="ps_c")
            for k in range(KC):
                cbk = (k % 2) * 128 + 32 * (k // 2)
                for j in range(G):
                    nc.tensor.matmul(
                        ps_c[32 * j:32 * (j + 1), :], rhT[:, cbk:cbk + B],
                        wc_sb[:, k * H + GW * j: k * H + GW * (j + 1)],
                        start=(k == 0), stop=(k == KC - 1),
                        skip_group_check=True, tile_position=(0, 32 * j))

            pre_c = work.tile([128, GW], F32, tag="pre_c")
            nc.vector.tensor_add(pre_c[:], ps_c[:], xc_t[:])
            c_sb = work.tile([128, GW], F32, tag="c")
            nc.scalar.activation(c_sb[:], pre_c[:],
                                 mybir.ActivationFunctionType.Tanh)

            drain(5)

            # --- h' = (h - c) * u + c ---
            d_sb = work.tile([128, GW], F32, tag="d")
            nc.vector.tensor_sub(d_sb[:], h_pk[:], c_sb[:])
            e_sb = work.tile([128, GW], F32, tag="e")
            nc.vector.tensor_mul(e_sb[:], d_sb[:], u_sb[:])
            h_new = hpk.tile([128, GW], F32, tag="h")
            nc.vector.tensor_add(h_new[:], e_sb[:], c_sb[:])

            # --- transpose h' into the staging block (k-major layout) ---
            ps_t2 = pst.tile([128, KC * B], F32, tag="ps_t")
            nc.tensor.transpose(ps_t2[:, 0:128], h_new[:, 0:128], ident[:])
            nc.tensor.transpose(ps_t2[:, 128:256], h_new[:, 128:256], ident[:])
            # psum col (two*128 + jj*32 + u) -> st col ((2*jj+two)*128 + sl*32 + u)
            src = ps_t2[:].rearrange("p (two jj u) -> p two jj u", two=2, jj=G)
            dst = st_blocks[m][:].rearrange(
                "p (jj two sl u) -> p two jj u sl", two=2, jj=G, sl=4)[:, :, :, :, sl]
            nc.vector.tensor_copy(dst, src)

            h_pk = h_new

            if sl == 3:
                queue_logits_block(m)

        for j in range(G):
            nc.sync.dma_start(hfin_d[:, GW * j:GW * (j + 1)],
                              h_pk[32 * j:32 * (j + 1), :])

        drain(len(pending))
        phase_a.close()

    nc.compile()
    return nc


_PROG_CACHE = {}


def _get_program(T):
    if T not in _PROG_CACHE:
        _PROG_CACHE[T] = build_program(T)
    return _PROG_CACHE[T]


def prep_tables(E_ru, b_ru, E_c, b_c):
    bf = ml_dtypes.bfloat16
    eall = np.concatenate([
        np.asarray(E_ru, np.float32) + np.asarray(b_ru, np.float32),
        np.asarray(E_c, np.float32) + np.asarray(b_c, np.float32),
    ], axis=1)
    return np.ascontiguousarray(eall.astype(bf))


def kernel(items, h0, E_ru, W_ru, b_ru, E_c, W_c, b_c, W_out):
    T = int(np.asarray(items).shape[1])
    items = np.ascontiguousarray(np.asarray(items).astype(np.int32))
    h0 = np.ascontiguousarray(np.asarray(h0, dtype=np.float32))
    bf = ml_dtypes.bfloat16
    eall = prep_tables(E_ru, b_ru, E_c, b_c)
    wru = np.asarray(W_ru, np.float32).astype(bf)
    wc = np.asarray(W_c, np.float32).astype(bf)
    wout_pad = np.zeros((H, NCORES * VSH), np.float32)
    wout_pad[:, :V] = np.asarray(W_out, np.float32)
    wout_pad = wout_pad.astype(bf)

    nc = _get_program(T)
    in_maps = []
    for c in range(NCORES):
        in_maps.append({
            "idx": items, "h0": h0, "eall": eall, "wru": wru, "wc": wc,
            "wout": np.ascontiguousarray(wout_pad[:, c * VSH:(c + 1) * VSH]),
        })
    res = run_bass_kernel_spmd(nc, in_maps, core_ids=list(range(NCORES)))
    h_final = res.results[0]["hfin"]
    logits = np.concatenate([res.results[c]["logits"] for c in range(NCORES)],
                            axis=1)[:, :V]
    return h_final, logits
